# revision 19
# baseline (speedup 1.0000x reference)
"""Trainium2 Bass kernel: multi-scale masked average-pool descriptors.

Computes, per batch element b and scribble i:
    d_l[b,i,c] = mean over {pixels where resize(scribble)[b,i,y,x] > 0.5} of feat_l[b,c,y,x]
    out[b,i,c] = (d_0 + d_1 + d_2) / 3

Key facts exploited:
  * jax.image.resize(bilinear, antialias=False) at scales 4/8/16 reduces to an
    exact 2x2 average at stride k with offset o (k,o) = (4,1)/(8,3)/(16,7):
    sr = 0.25*((a+c)+(b+d)) bit-exactly.  So mask == ((a+c)+(b+d)) > 2.0 with the
    same fp32 association -> masks match the reference bit-exactly.  Only rows
    {o, o+1 mod k} x cols {o, o+1 mod k} of the scribbles participate, so the
    host stages exactly those (pure gather): 5.5MB instead of 16.8MB, fp32.
  * The masked sum is a matmul over pixels: ssum[i,c] = sum_s maskT[s,i]*f[s,c].
    The host pre-transposes each feature level to [y, x, C+1] (pure layout) and
    appends a ones column, so the device consumes features with fully
    contiguous >=8KB DMA descriptors, and cnt[i] falls out of the same matmul
    as column C (exact: 0/1 * 1 accumulated in fp32 PSUM).
  * Features are host-cast to fp8e4 (level 0) / bf16 (levels 1, 2): masked
    means average ~512-8192 elements, so quantization noise averages down
    (measured end-to-end rel err ~7e-3 vs the 2e-2 gate).  Masks are computed
    exactly in fp32 on DVE and written in the level's dtype (0/1 exact).
  * Per x-column matmul: lhsT = mask column [K=h, 16], rhs = features [K=h,
    257] -> PSUM acc [16, 257] accumulated across all x.  Level 0 is split
    into 4 x-chunks so its masks/matmuls pipeline with the DMA stream.
  * The empty-mask fallback is handled on the host (it never triggers for
    non-degenerate inputs; P(empty mask) <= 2^-1024).

Sharding: pure data-parallel over batch B=8 across the 8 NeuronCores.
"""

import numpy as np

_B = 8
_I = 16
_C = 256
_N = _C + 1  # channels + ones column (count)

# level config: li -> (h, k, off).  Levels with h < 128 pack pk = 128//h
# x-columns into the partition dim (partition p = d*h + y for x = pk*xh + d)
# so every DMA spans all 128 partitions and every matmul contracts K=128.
_LEVELS = {
    0: (128, 4, 1),
    1: (64, 8, 3),
    2: (32, 16, 7),
}
_XC0 = (40, 40, 32, 16)  # level-0 x-chunk widths (last smallest: it gates the tail)
_NCH0 = len(_XC0)
# merged "head" transfer (one fat descriptor per partition instead of three
# small ones): f32 units per partition region
_HF2 = 8 * _N // 2  # ft2 [128, 8, 257] bf16 viewed as f32 pairs
_HS2 = _I * 32  # st2 [128, 16, 32] f32
_HS1 = _I * 128  # st1 [128, 16, 128] f32
_HEAD = _HF2 + _HS2 + _HS1


def _ml_dtypes():
    try:
        import ml_dtypes
    except ImportError:
        import sys

        for p in ("/opt/trn_rl_repo", "/opt/pypackages"):
            if p not in sys.path:
                sys.path.append(p)
        import ml_dtypes
    return ml_dtypes


def _build_nc():
    import concourse.bacc as bacc
    import concourse.tile as tile
    from concourse import mybir

    f32 = mybir.dt.float32
    bf16 = mybir.dt.bfloat16
    fp8 = mybir.dt.float8e4
    gt = mybir.AluOpType.is_gt

    nc = bacc.Bacc("TRN2", target_bir_lowering=False, debug=False)

    headd = nc.dram_tensor("head", [128, _HEAD], f32, kind="ExternalInput")
    s0d = [
        nc.dram_tensor(f"s0c{j}", [128, _I, 4 * wc], f32, kind="ExternalInput")
        for j, wc in enumerate(_XC0)
    ]
    f1d = nc.dram_tensor("f1", [128, 32, _N], bf16, kind="ExternalInput")
    f0d = nc.dram_tensor("f0", [128, 128, _N], fp8, kind="ExternalInput")
    out_d = nc.dram_tensor("out", [_I, 3 * _N], f32, kind="ExternalOutput")

    with tile.TileContext(nc) as tc:
        with (
            tc.tile_pool(name="singles", bufs=1) as singles,
            tc.tile_pool(name="vtmp", bufs=2) as vtmp,
            tc.tile_pool(name="htmp", bufs=2) as htmp,
            tc.tile_pool(name="psum", bufs=3, space="PSUM") as psum,
        ):
            stag = singles.tile([_I, 3 * _N], f32, tag="stag")

            # ---- PE warm-up ---------------------------------------------
            # The PE clock-gate (HAM) runs at 1.2 GHz until ~3.4us of
            # sustained activity; dummy matmuls during the DMA ramp warm it
            # so the real matmuls all run at 2.4 GHz.
            warm = singles.tile([128, 512], bf16, tag="warm")
            nc.vector.memset(warm[:], 0.0)
            wacc = psum.tile([_I, 512], f32, tag="wacc")
            for _ in range(10):
                nc.tensor.matmul(
                    wacc[:], warm[:, 0:_I], warm[:], start=True, stop=True
                )

            # ---- DMA emission -------------------------------------------
            # Two HWDGE rings only (SWDGE bulk measured a flat ~105 GB/s and
            # dragged both HWDGE rings down).  Descriptor size rules the per-
            # queue rate (~52 GB/s @2KB/partition, ~285 @8KB, ~330 @16KB), so
            # the three small head transfers ride one merged descriptor.
            # sync: head (f2+s2+s1) + s0 chunks; scalar: f1 + f0 chunks.
            head = singles.tile([128, _HEAD], f32, tag="head")
            nc.sync.dma_start(out=head[:], in_=headd[:])
            ft2 = head[:, 0:_HF2].bitcast(bf16).rearrange("p (x c) -> p x c", c=_N)
            st2 = head[:, _HF2 : _HF2 + _HS2].rearrange("p (i w) -> p i w", i=_I)
            st1 = head[:, _HF2 + _HS2 : _HEAD].rearrange("p (i w) -> p i w", i=_I)
            st0 = []
            for j, wc in enumerate(_XC0):
                t = singles.tile([128, _I, 4 * wc], f32, tag=f"st0{j}")
                nc.sync.dma_start(out=t[:], in_=s0d[j][:])
                st0.append(t)

            ft1 = []
            for j in range(2):
                t = singles.tile([128, 16, _N], bf16, tag=f"ft1{j}")
                nc.scalar.dma_start(out=t[:], in_=f1d[:, j * 16 : (j + 1) * 16, :])
                ft1.append(t)
            ft0 = []
            x0 = 0
            for j, wc in enumerate(_XC0):
                t = singles.tile([128, wc, _N], fp8, tag=f"ft0{j}")
                nc.scalar.dma_start(out=t[:], in_=f0d[:, x0 : x0 + wc, :])
                ft0.append(t)
                x0 += wc

            def make_mask(st, h, w, dt, tag):
                """st: [h, I, 4w] = (row0 cols | row1 cols) -> mask [h, I, w]."""
                v = vtmp.tile([h, _I, 2 * w], f32, tag=f"v{w}")
                nc.vector.tensor_add(v[:], st[:, :, 0 : 2 * w], st[:, :, 2 * w : 4 * w])
                hh = htmp.tile([h, _I, w], f32, tag=f"h{w}")
                vp = v[:].rearrange("p i (x k) -> p i x k", k=2)
                nc.vector.tensor_add(hh[:], vp[:, :, :, 0], vp[:, :, :, 1])
                m = singles.tile([h, _I, w], dt, tag=tag)
                nc.vector.tensor_scalar(
                    out=m[:], in0=hh[:], scalar1=2.0, scalar2=None, op0=gt
                )
                return m

            def level_matmuls(m, ft, acc, w, first, last):
                for x in range(w):
                    nc.tensor.matmul(
                        acc[:],
                        m[:, :, x],
                        ft[:, x, :],
                        start=(first and x == 0),
                        stop=(last and x == w - 1),
                    )

            # ---- level 2 then 1 then 0 (chunked) ------------------------
            # DVE queue carries only mask production; PSUM->SBUF staging
            # copies ride the scalar engine so they never head-of-line
            # block later masks.
            m2 = make_mask(st2, 128, 8, bf16, "m2")
            m1 = make_mask(st1, 128, 32, bf16, "m1")

            acc2 = psum.tile([_I, _N], f32, tag="acc")
            level_matmuls(m2, ft2, acc2, 8, True, True)
            nc.scalar.copy(stag[:, 2 * _N : 3 * _N], acc2[:])
            nc.scalar.dma_start(
                out=out_d[:, 2 * _N : 3 * _N], in_=stag[:, 2 * _N : 3 * _N]
            )

            acc1 = psum.tile([_I, _N], f32, tag="acc")
            for j in range(2):
                level_matmuls(
                    m1[:, :, j * 16 : (j + 1) * 16], ft1[j], acc1, 16, j == 0, j == 1
                )
            nc.scalar.copy(stag[:, _N : 2 * _N], acc1[:])
            nc.scalar.dma_start(out=out_d[:, _N : 2 * _N], in_=stag[:, _N : 2 * _N])

            acc0 = psum.tile([_I, _N], f32, tag="acc")
            for j, wc in enumerate(_XC0):
                m0 = make_mask(st0[j], 128, wc, fp8, f"m0{j}")
                level_matmuls(m0, ft0[j], acc0, wc, j == 0, j == _NCH0 - 1)
            nc.scalar.copy(stag[:, 0:_N], acc0[:])
            nc.scalar.dma_start(out=out_d[:, 0:_N], in_=stag[:, 0:_N])

    nc.compile()
    return nc


def _stage_feat(f, np_dt):
    """[C, h, w] fp32 -> [128, h*w//128, C+1] in np_dt with ones column.

    Partition p = d*h + y for x = pk*xh + d (pk = 128//h): free dims (xh, c).
    """
    h = f.shape[1]
    pk = 128 // h
    wh = h // pk
    out = np.empty((128, wh, _N), dtype=np_dt)
    # [C,h,w] -> [h, xh, d, C] -> [d, h, xh, C] -> [128, wh, C]
    t = f.transpose(1, 2, 0).reshape(h, wh, pk, _C).transpose(2, 0, 1, 3)
    out[:, :, :_C] = t.reshape(128, wh, _C).astype(np_dt)
    out[:, :, _C] = np.asarray(1.0, dtype=np_dt)
    return out


def _stage_scr(scr, h, k, off, widths=None):
    """[I, 512, 512] fp32 -> list of [128, I, 4*wc] gathered row/col pairs in
    the same packed-partition layout as _stage_feat:
    s[d*h+y, i, :] = (row k*y+off cols | row k*y+off+1 cols) for x = pk*xh+d.
    """
    pk = 128 // h
    wh = h // pk  # xh count (before chunking)
    idx = (np.arange(h)[:, None] * k + off + np.arange(2)[None, :]).ravel()
    g = scr[:, idx][:, :, idx]  # [I, 2h, 2w]
    # cols axis 2w: x = pk*xh + d, pair cc -> [I, y, r, xh, d, cc]
    g = g.reshape(_I, h, 2, wh, pk, 2)
    # -> [d, y, I, r, xh, cc]
    g = g.transpose(4, 1, 0, 2, 3, 5)
    outs, x0 = [], 0
    for wc in widths if widths is not None else [wh]:
        outs.append(
            np.ascontiguousarray(
                g[:, :, :, :, x0 : x0 + wc, :].reshape(128, _I, 4 * wc)
            )
        )
        x0 += wc
    return outs


def _in_map(inputs, b):
    md = _ml_dtypes()
    bf16 = md.bfloat16
    fp8 = md.float8_e4m3
    scr = np.asarray(inputs["scribbles"][b], np.float32)
    ft2 = _stage_feat(np.asarray(inputs["feat2"][b], np.float32), bf16)
    st2 = _stage_scr(scr, 32, 16, 7)[0]
    st1 = _stage_scr(scr, 64, 8, 3)[0]
    head = np.empty((128, _HEAD), np.float32)
    head[:, 0:_HF2] = ft2.reshape(128, 8 * _N).view(np.float32)
    head[:, _HF2 : _HF2 + _HS2] = st2.reshape(128, _HS2)
    head[:, _HF2 + _HS2 :] = st1.reshape(128, _HS1)
    m = {
        "head": head,
        "f0": _stage_feat(np.asarray(inputs["feat0"][b], np.float32), fp8),
        "f1": _stage_feat(np.asarray(inputs["feat1"][b], np.float32), bf16),
    }
    for j, a in enumerate(_stage_scr(scr, 128, 4, 1, widths=_XC0)):
        m[f"s0c{j}"] = a
    return m


def _host_fallback(scr_bi, fmap_b, h, k, off):
    """Feature at argmax of the soft mask; only used when a mask is empty."""
    V = scr_bi[off::k, :][:h].astype(np.float32) + scr_bi[off + 1 :: k, :][:h]
    sr4 = V[:, off::k][:, :h] + V[:, off + 1 :: k][:, :h]
    idx = int(np.argmax(np.float32(0.25) * sr4))
    y, x = divmod(idx, h)
    return fmap_b[:, y, x]


def kernel(feat0, feat1, feat2, scribbles):
    import sys

    for p in ("/opt/trn_rl_repo", "/opt/pypackages"):
        if p not in sys.path:
            sys.path.append(p)
    from concourse.bass_utils import run_bass_kernel_spmd

    inputs = {
        "feat0": np.asarray(feat0, dtype=np.float32),
        "feat1": np.asarray(feat1, dtype=np.float32),
        "feat2": np.asarray(feat2, dtype=np.float32),
        "scribbles": np.asarray(scribbles, dtype=np.float32),
    }
    feat0, feat1, feat2, scribbles = (
        inputs["feat0"], inputs["feat1"], inputs["feat2"], inputs["scribbles"]
    )

    nc = _build_nc()
    in_maps = [_in_map(inputs, b) for b in range(_B)]
    res = run_bass_kernel_spmd(nc, in_maps, core_ids=list(range(_B)))
    raw = np.stack([res.results[b]["out"] for b in range(_B)])  # [B, I, 3*257]
    raw = raw.reshape(_B, _I, 3, _N)
    ssum = raw[..., :_C].astype(np.float32)  # [B, I, 3, C]
    cnt = raw[..., _C].astype(np.float32)  # [B, I, 3]

    mean = ssum / np.maximum(cnt, np.float32(1.0))[..., None]

    if (cnt == 0).any():  # never for non-degenerate inputs
        fm = [feat0, feat1, feat2]
        for b, i, li in zip(*np.nonzero(cnt == 0)):
            h, k, off = _LEVELS[li]
            mean[b, i, li] = _host_fallback(scribbles[b, i], fm[li][b], h, k, off)

    out = (mean[:, :, 0] + mean[:, :, 1] + mean[:, :, 2]) / np.float32(3.0)
    return out.astype(np.float32)


# revision 22
# speedup vs baseline: 1.0331x; 1.0331x over previous
"""Trainium2 Bass kernel: multi-scale masked average-pool descriptors.

Computes, per batch element b and scribble i:
    d_l[b,i,c] = mean over {pixels where resize(scribble)[b,i,y,x] > 0.5} of feat_l[b,c,y,x]
    out[b,i,c] = (d_0 + d_1 + d_2) / 3

Key facts exploited:
  * jax.image.resize(bilinear, antialias=False) at scales 4/8/16 reduces to an
    exact 2x2 average at stride k with offset o (k,o) = (4,1)/(8,3)/(16,7):
    sr = 0.25*((a+c)+(b+d)) bit-exactly.  So mask == ((a+c)+(b+d)) > 2.0 with the
    same fp32 association -> masks match the reference bit-exactly.  Only rows
    {o, o+1 mod k} x cols {o, o+1 mod k} of the scribbles participate, so the
    host stages exactly those (pure gather): 5.5MB instead of 16.8MB, fp32.
  * The masked sum is a matmul over pixels: ssum[i,c] = sum_s maskT[s,i]*f[s,c].
    The host pre-transposes each feature level to [y, x, C+1] (pure layout) and
    appends a ones column, so the device consumes features with fully
    contiguous >=8KB DMA descriptors, and cnt[i] falls out of the same matmul
    as column C (exact: 0/1 * 1 accumulated in fp32 PSUM).
  * Features are host-cast to fp8e4 (level 0) / bf16 (levels 1, 2): masked
    means average ~512-8192 elements, so quantization noise averages down
    (measured end-to-end rel err ~7e-3 vs the 2e-2 gate).  Masks are computed
    exactly in fp32 on DVE and written in the level's dtype (0/1 exact).
  * Per x-column matmul: lhsT = mask column [K=h, 16], rhs = features [K=h,
    257] -> PSUM acc [16, 257] accumulated across all x.  Level 0 is split
    into 4 x-chunks so its masks/matmuls pipeline with the DMA stream.
  * The empty-mask fallback is handled on the host (it never triggers for
    non-degenerate inputs; P(empty mask) <= 2^-1024).

Sharding: pure data-parallel over batch B=8 across the 8 NeuronCores.
"""

import numpy as np

_B = 8
_I = 16
_C = 256
_N = _C + 1  # channels + ones column (count)

# level config: li -> (h, k, off).  Levels with h < 128 pack pk = 128//h
# x-columns into the partition dim (partition p = d*h + y for x = pk*xh + d)
# so every DMA spans all 128 partitions and every matmul contracts K=128.
_LEVELS = {
    0: (128, 4, 1),
    1: (64, 8, 3),
    2: (32, 16, 7),
}
_XC0 = (32, 32, 32, 32)  # level-0 x-chunk widths
_NCH0 = len(_XC0)
# merged "head" transfer (one fat descriptor per partition instead of three
# small ones): f32 units per partition region
_HF2 = 8 * _N // 2  # ft2 [128, 8, 257] bf16 viewed as f32 pairs
_HS2 = _I * 32  # st2 [128, 16, 32] f32
_HS1 = _I * 128  # st1 [128, 16, 128] f32
_HEAD = _HF2 + _HS2 + _HS1


def _ml_dtypes():
    try:
        import ml_dtypes
    except ImportError:
        import sys

        for p in ("/opt/trn_rl_repo", "/opt/pypackages"):
            if p not in sys.path:
                sys.path.append(p)
        import ml_dtypes
    return ml_dtypes


def _build_nc():
    import concourse.bacc as bacc
    import concourse.tile as tile
    from concourse import mybir

    f32 = mybir.dt.float32
    bf16 = mybir.dt.bfloat16
    fp8 = mybir.dt.float8e4
    gt = mybir.AluOpType.is_gt

    nc = bacc.Bacc("TRN2", target_bir_lowering=False, debug=False)

    headd = nc.dram_tensor("head", [128, _HEAD], f32, kind="ExternalInput")
    s0d = [
        nc.dram_tensor(f"s0c{j}", [128, _I, 4 * wc], f32, kind="ExternalInput")
        for j, wc in enumerate(_XC0)
    ]
    f1d = nc.dram_tensor("f1", [128, 32, _N], bf16, kind="ExternalInput")
    f0d = nc.dram_tensor("f0", [128, 128, _N], fp8, kind="ExternalInput")
    out_d = nc.dram_tensor("out", [_I, 3 * _N], f32, kind="ExternalOutput")

    with tile.TileContext(nc) as tc:
        with (
            tc.tile_pool(name="singles", bufs=1) as singles,
            tc.tile_pool(name="vtmp", bufs=2) as vtmp,
            tc.tile_pool(name="htmp", bufs=2) as htmp,
            tc.tile_pool(name="psum", bufs=3, space="PSUM") as psum,
        ):
            stag = singles.tile([_I, 3 * _N], f32, tag="stag")

            # ---- PE warm-up ---------------------------------------------
            # The PE clock-gate (HAM) runs at 1.2 GHz until ~3.4us of
            # sustained activity; dummy matmuls during the DMA ramp warm it
            # so the real matmuls all run at 2.4 GHz.
            warm = singles.tile([128, 512], bf16, tag="warm")
            nc.vector.memset(warm[:], 0.0)
            wacc = psum.tile([_I, 512], f32, tag="wacc")
            for _ in range(10):
                nc.tensor.matmul(
                    wacc[:], warm[:, 0:_I], warm[:], start=True, stop=True
                )

            # ---- DMA emission -------------------------------------------
            # Two HWDGE rings only (SWDGE bulk measured a flat ~105 GB/s and
            # dragged both HWDGE rings down).  Descriptor size rules the per-
            # queue rate (~52 GB/s @2KB/partition, ~285 @8KB, ~330 @16KB), so
            # the three small head transfers ride one merged descriptor.
            # sync: head (f2+s2+s1) + s0 chunks; scalar: f1 + f0 chunks.
            head = singles.tile([128, _HEAD], f32, tag="head")
            nc.sync.dma_start(out=head[:], in_=headd[:])
            ft2 = head[:, 0:_HF2].bitcast(bf16).rearrange("p (x c) -> p x c", c=_N)
            st2 = head[:, _HF2 : _HF2 + _HS2].rearrange("p (i w) -> p i w", i=_I)
            st1 = head[:, _HF2 + _HS2 : _HEAD].rearrange("p (i w) -> p i w", i=_I)
            st0 = []
            for j, wc in enumerate(_XC0):
                t = singles.tile([128, _I, 4 * wc], f32, tag=f"st0{j}")
                nc.sync.dma_start(out=t[:], in_=s0d[j][:])
                st0.append(t)

            ft1 = singles.tile([128, 32, _N], bf16, tag="ft1")
            nc.scalar.dma_start(out=ft1[:], in_=f1d[:])
            ft0 = []
            x0 = 0
            for j, wc in enumerate(_XC0):
                t = singles.tile([128, wc, _N], fp8, tag=f"ft0{j}")
                nc.scalar.dma_start(out=t[:], in_=f0d[:, x0 : x0 + wc, :])
                ft0.append(t)
                x0 += wc

            def make_mask(st, h, w, dt, tag):
                """st: [h, I, 4w] = (row0 cols | row1 cols) -> mask [h, I, w]."""
                v = vtmp.tile([h, _I, 2 * w], f32, tag=f"v{w}")
                nc.vector.tensor_add(v[:], st[:, :, 0 : 2 * w], st[:, :, 2 * w : 4 * w])
                hh = htmp.tile([h, _I, w], f32, tag=f"h{w}")
                vp = v[:].rearrange("p i (x k) -> p i x k", k=2)
                nc.vector.tensor_add(hh[:], vp[:, :, :, 0], vp[:, :, :, 1])
                m = singles.tile([h, _I, w], dt, tag=tag)
                nc.vector.tensor_scalar(
                    out=m[:], in0=hh[:], scalar1=2.0, scalar2=None, op0=gt
                )
                return m

            def level_matmuls(m, ft, acc, w, first, last):
                for x in range(w):
                    nc.tensor.matmul(
                        acc[:],
                        m[:, :, x],
                        ft[:, x, :],
                        start=(first and x == 0),
                        stop=(last and x == w - 1),
                    )

            # ---- level 2 then 1 then 0 (chunked) ------------------------
            # DVE queue carries only mask production; PSUM->SBUF staging
            # copies ride the scalar engine so they never head-of-line
            # block later masks.
            m2 = make_mask(st2, 128, 8, bf16, "m2")
            m1 = make_mask(st1, 128, 32, bf16, "m1")

            acc2 = psum.tile([_I, _N], f32, tag="acc")
            level_matmuls(m2, ft2, acc2, 8, True, True)
            nc.scalar.copy(stag[:, 2 * _N : 3 * _N], acc2[:])
            nc.scalar.dma_start(
                out=out_d[:, 2 * _N : 3 * _N], in_=stag[:, 2 * _N : 3 * _N]
            )

            # filler while ft1 streams in: keeps the PE clock-gate warm
            for _ in range(14):
                nc.tensor.matmul(
                    wacc[:], warm[:, 0:_I], warm[:], start=True, stop=True
                )

            acc1 = psum.tile([_I, _N], f32, tag="acc")
            level_matmuls(m1, ft1, acc1, 32, True, True)
            nc.scalar.copy(stag[:, _N : 2 * _N], acc1[:])
            nc.scalar.dma_start(out=out_d[:, _N : 2 * _N], in_=stag[:, _N : 2 * _N])

            acc0 = psum.tile([_I, _N], f32, tag="acc")
            for j, wc in enumerate(_XC0):
                m0 = make_mask(st0[j], 128, wc, fp8, f"m0{j}")
                level_matmuls(m0, ft0[j], acc0, wc, j == 0, j == _NCH0 - 1)
            nc.scalar.copy(stag[:, 0:_N], acc0[:])
            nc.scalar.dma_start(out=out_d[:, 0:_N], in_=stag[:, 0:_N])

    nc.compile()
    return nc


def _stage_feat(f, np_dt):
    """[C, h, w] fp32 -> [128, h*w//128, C+1] in np_dt with ones column.

    Partition p = d*h + y for x = pk*xh + d (pk = 128//h): free dims (xh, c).
    """
    h = f.shape[1]
    pk = 128 // h
    wh = h // pk
    out = np.empty((128, wh, _N), dtype=np_dt)
    # [C,h,w] -> [h, xh, d, C] -> [d, h, xh, C] -> [128, wh, C]
    t = f.transpose(1, 2, 0).reshape(h, wh, pk, _C).transpose(2, 0, 1, 3)
    out[:, :, :_C] = t.reshape(128, wh, _C).astype(np_dt)
    out[:, :, _C] = np.asarray(1.0, dtype=np_dt)
    return out


def _stage_scr(scr, h, k, off, widths=None):
    """[I, 512, 512] fp32 -> list of [128, I, 4*wc] gathered row/col pairs in
    the same packed-partition layout as _stage_feat:
    s[d*h+y, i, :] = (row k*y+off cols | row k*y+off+1 cols) for x = pk*xh+d.
    """
    pk = 128 // h
    wh = h // pk  # xh count (before chunking)
    idx = (np.arange(h)[:, None] * k + off + np.arange(2)[None, :]).ravel()
    g = scr[:, idx][:, :, idx]  # [I, 2h, 2w]
    # cols axis 2w: x = pk*xh + d, pair cc -> [I, y, r, xh, d, cc]
    g = g.reshape(_I, h, 2, wh, pk, 2)
    # -> [d, y, I, r, xh, cc]
    g = g.transpose(4, 1, 0, 2, 3, 5)
    outs, x0 = [], 0
    for wc in widths if widths is not None else [wh]:
        outs.append(
            np.ascontiguousarray(
                g[:, :, :, :, x0 : x0 + wc, :].reshape(128, _I, 4 * wc)
            )
        )
        x0 += wc
    return outs


def _in_map(inputs, b):
    md = _ml_dtypes()
    bf16 = md.bfloat16
    fp8 = md.float8_e4m3
    scr = np.asarray(inputs["scribbles"][b], np.float32)
    ft2 = _stage_feat(np.asarray(inputs["feat2"][b], np.float32), bf16)
    st2 = _stage_scr(scr, 32, 16, 7)[0]
    st1 = _stage_scr(scr, 64, 8, 3)[0]
    head = np.empty((128, _HEAD), np.float32)
    head[:, 0:_HF2] = ft2.reshape(128, 8 * _N).view(np.float32)
    head[:, _HF2 : _HF2 + _HS2] = st2.reshape(128, _HS2)
    head[:, _HF2 + _HS2 :] = st1.reshape(128, _HS1)
    m = {
        "head": head,
        "f0": _stage_feat(np.asarray(inputs["feat0"][b], np.float32), fp8),
        "f1": _stage_feat(np.asarray(inputs["feat1"][b], np.float32), bf16),
    }
    for j, a in enumerate(_stage_scr(scr, 128, 4, 1, widths=_XC0)):
        m[f"s0c{j}"] = a
    return m


def _host_fallback(scr_bi, fmap_b, h, k, off):
    """Feature at argmax of the soft mask; only used when a mask is empty."""
    V = scr_bi[off::k, :][:h].astype(np.float32) + scr_bi[off + 1 :: k, :][:h]
    sr4 = V[:, off::k][:, :h] + V[:, off + 1 :: k][:, :h]
    idx = int(np.argmax(np.float32(0.25) * sr4))
    y, x = divmod(idx, h)
    return fmap_b[:, y, x]


def kernel(feat0, feat1, feat2, scribbles):
    import sys

    for p in ("/opt/trn_rl_repo", "/opt/pypackages"):
        if p not in sys.path:
            sys.path.append(p)
    from concourse.bass_utils import run_bass_kernel_spmd

    inputs = {
        "feat0": np.asarray(feat0, dtype=np.float32),
        "feat1": np.asarray(feat1, dtype=np.float32),
        "feat2": np.asarray(feat2, dtype=np.float32),
        "scribbles": np.asarray(scribbles, dtype=np.float32),
    }
    feat0, feat1, feat2, scribbles = (
        inputs["feat0"], inputs["feat1"], inputs["feat2"], inputs["scribbles"]
    )

    nc = _build_nc()
    in_maps = [_in_map(inputs, b) for b in range(_B)]
    res = run_bass_kernel_spmd(nc, in_maps, core_ids=list(range(_B)))
    raw = np.stack([res.results[b]["out"] for b in range(_B)])  # [B, I, 3*257]
    raw = raw.reshape(_B, _I, 3, _N)
    ssum = raw[..., :_C].astype(np.float32)  # [B, I, 3, C]
    cnt = raw[..., _C].astype(np.float32)  # [B, I, 3]

    mean = ssum / np.maximum(cnt, np.float32(1.0))[..., None]

    if (cnt == 0).any():  # never for non-degenerate inputs
        fm = [feat0, feat1, feat2]
        for b, i, li in zip(*np.nonzero(cnt == 0)):
            h, k, off = _LEVELS[li]
            mean[b, i, li] = _host_fallback(scribbles[b, i], fm[li][b], h, k, off)

    out = (mean[:, :, 0] + mean[:, :, 1] + mean[:, :, 2]) / np.float32(3.0)
    return out.astype(np.float32)
